# revision 50
# baseline (speedup 1.0000x reference)
"""MoE top-2 routed linear (nn_MoELinear) on 8 Trainium2 NeuronCores.

Strategy (load-balanced expert parallelism, fp16):
  - Gating (tiny: [N,1024]x[1024,8] matmul + top-2 + softmax) runs on host
    with jax-CPU, replicating the reference op-for-op so the top-2 decisions
    match the reference bitwise.  Gate weights are folded into x on host
    (x is fp16, a relative format, so the scaling costs no precision).
  - The per-expert work is split into "jobs" = (expert, 2048-col output
    half).  Each core holds up to 4 resident W segments of [1024, 2048]
    (fp16) in SBUF; a job is assigned to one or more (core, segment) cells,
    and its token tiles (128 tokens each) fill that cell's slot range.  A
    small search over segment capacities packs all jobs into the minimum
    uniform slot count MT per core (= ceil(2*sum(M_e)/8) = 33 for balanced
    routing), eliminating the load imbalance of one-expert-per-core
    (MT-equivalent 36) -- the PE stream is the bottleneck, so slots are
    roughly proportional to exec time.
  - Per slot the device runs 4 psum groups x 8 fp16 matmuls (128x128x512,
    fp32 PSUM accumulation), evicts through the vector engine as a plain
    cast to fp16, and stores y via scalar-triggered DMA.  Steady-state
    slots use n-inner matmul order so 3/4 of the LDWEIGHTS reload an
    identical stationary operand; post-finalize BIR passes delete those
    (1056 -> ~320 LDWEIGHTS on HW) and strip the per-matmul completion sem
    posts down to the stop-matmuls (remapping wait thresholds).  Both are
    verified correct; pair time stays ~222ns = 512 stream cycles + ~21
    cycles of PE issue cost, so they only trim queue/sem pressure.  W rides the sync queue as big
    multi-dim descriptors (the ~0.6us/descriptor trigger cost and
    ring-depth throttling otherwise stretch W delivery); the first two
    n-groups are split per-k across sync+scalar so the first slot can start
    after ~128KB, and the first R=6 slots' psum groups interleave n-major
    (they share seg0's chunks, so each chunk is needed R slots later --
    this hid the seg0 delivery ramp entirely: mm-stream gaps 6.8us ->
    1.8us, 259us -> 252.7us).  x tiles prefetch 6 deep on gpsimd.  Engine separation
    matters: any long-waiting descriptor queued on an engine head-of-line
    blocks everything behind it (measured as psum-eviction convoys).
  - Host scatters/accumulates the per-slot fp16 outputs into the final
    fp32 result.  Measured max rel err ~4.4e-4 vs the 2e-2 gate.
"""

import os

import numpy as np

NUM_CORES = 8
NUM_EXPERTS = 8
TOP_K = 2
P = 128  # partitions
N_TILE = 512  # psum free-dim tile (one bank of fp32)
SEG_COLS = 2048  # output columns per W segment (half of d_out)
MAX_SEGS = 4

# enable NTFF tracing (sets LAST_RUN_INFO["exec_time_ns"])
TRACE = os.environ.get("MOE_TRACE", "0") == "1"
# n-inner matmul ordering + redundant-LDWEIGHTS removal
LDW_ELIDE = os.environ.get("MOE_LDW_ELIDE", "1") == "1"
# strip per-matmul completion sem posts (keep only stop-matmuls')
SEM_ELIDE = os.environ.get("MOE_SEM_ELIDE", "1") == "1"
MM_DTYPE = "float16"

LAST_RUN_INFO = {}
_NC_CACHE = {}


def _routing(x_flat, Wg, bg):
    """Replicate the reference gating bitwise on jax-CPU; numpy fallback."""
    try:
        import jax
        import jax.numpy as jnp

        with jax.default_device(jax.devices("cpu")[0]):
            xf = jnp.asarray(x_flat)
            gate_logits = xf @ jnp.asarray(Wg).T + jnp.asarray(bg)
            top_w, top_idx = jax.lax.top_k(gate_logits, TOP_K)
            top_w = jax.nn.softmax(top_w, axis=-1)
            return np.asarray(top_idx), np.asarray(top_w)
    except Exception:
        logits = x_flat @ Wg.T + bg
        top_idx = np.argsort(-logits, axis=1, kind="stable")[:, :TOP_K]
        top_v = np.take_along_axis(logits, top_idx, axis=1)
        e = np.exp(top_v - top_v.max(axis=1, keepdims=True))
        top_w = e / e.sum(axis=1, keepdims=True)
        return top_idx, top_w.astype(np.float32)


def _try_assign(jobs, caps):
    """Greedy pack of jobs (size, e, h) into cells of capacities caps x 8.

    Returns list of (e, h, tile_lo, tile_hi, core, seg) cell fills, or None.
    A job may span several cells (its tiles are split across them)."""
    cells = []  # [remaining?, core, seg, cap]
    for s, q in enumerate(caps):
        if q <= 0:
            continue
        for core in range(NUM_CORES):
            cells.append([q, core, s, False])  # cap, core, seg, used
    fills = []
    for size, e, h in jobs:
        free = [c for c in cells if not c[3]]
        # smallest single cell that fits
        fit = None
        for c in sorted(free, key=lambda c: c[0]):
            if c[0] >= size:
                fit = c
                break
        take = []
        if fit is not None:
            take = [fit]
        else:
            got = 0
            for c in sorted(free, key=lambda c: -c[0]):
                if c in take:
                    continue
                # if a single remaining cell can finish the cover, use the
                # smallest such cell instead of the next-largest
                fin = None
                for c2 in sorted(free, key=lambda c: c[0]):
                    if c2 not in take and c2[0] >= size - got:
                        fin = c2
                        break
                if fin is not None:
                    take.append(fin)
                    got += fin[0]
                    break
                take.append(c)
                got += c[0]
                if got >= size:
                    break
            if got < size:
                return None
        lo = 0
        for c in take:
            c[3] = True
            n = min(c[0], size - lo)
            fills.append((e, h, lo, lo + n, c[1], c[2]))
            lo += n
    return fills


def _plan(M):
    """Choose slot count MT, segment capacities, and the job->cell packing."""
    jobs = []
    for e in range(NUM_EXPERTS):
        if M[e] > 0:
            jobs.append((M[e], e, 0))
            jobs.append((M[e], e, 1))
    jobs.sort(reverse=True)
    total = sum(s for s, _, _ in jobs)
    lb = max(1, -(-total // NUM_CORES))
    for mt in range(lb, 4 * 64 + 1):
        for a in range(-(-mt // MAX_SEGS), mt + 1):
            for b in range(0, min(a, mt - a) + 1):
                rem = mt - a - b
                for c in range(max(0, rem - b), min(b, rem) + 1):
                    dd = rem - c
                    if dd > c:
                        continue
                    caps = (a, b, c, dd)
                    fills = _try_assign(jobs, caps)
                    if fills is not None:
                        return mt, caps, fills
    raise RuntimeError("no feasible packing")


def _elide_redundant_ldweights(nc):
    """Post-finalize BIR pass: drop InstLdweights that reload the identical
    weights AP as the previous one (n-inner matmul runs) and carry no
    waits/updates; mark all matmuls non-self-loading so walrus pairs them
    with the surviving loads."""
    import concourse.mybir as mybir

    removed = 0
    for f in nc.m.functions:
        for b in f.blocks:
            new = []
            loaded = None
            for i in b.instructions:
                tn = type(i).__name__
                if tn == "InstLdweights":
                    key = repr(i.ins[0])
                    if loaded == key and not i.has_wait() and not i.has_update():
                        removed += 1
                        continue
                    loaded = key
                elif tn == "InstMatmult":
                    i.ldweights = False
                new.append(i)
            b.instructions = new
    return removed


def _elide_matmul_sem_posts(nc):
    """Post-finalize BIR pass: matmuls post a completion increment on a
    shared counter sem, but every wait threshold on it lands on an
    accumulation-group stop boundary (PE is in-order).  Keep the post only
    on stop matmuls and remap wait thresholds from mm-counts to
    stop-counts.  Bails out (returns 0) unless the sync graph matches the
    expected shape exactly."""
    import concourse.mybir as mybir

    mms = []
    for f in nc.m.functions:
        for b in f.blocks:
            for i in b.instructions:
                if type(i).__name__ == "InstMatmult":
                    mms.append(i)
    if not mms:
        return 0
    # the counter sem = the one updated by every matmul with sem-inc
    from collections import Counter

    upd = Counter()
    for i in mms:
        for u in i.sync_info.on_update:
            if u.update_mode == "sem-inc" and u.update_value == 1:
                upd[(u.sync_type, u.id)] += 1
    keys = [k for k, c in upd.items() if c == len(mms)]
    if len(keys) != 1:
        return 0
    skey = keys[0]

    stop_flags = [bool(i.stop_tensor_calc) for i in mms]
    prefix = [0]
    for fl in stop_flags:
        prefix.append(prefix[-1] + (1 if fl else 0))

    def remap(x):
        x = min(x, len(mms))
        if x <= 0:
            return 0
        return prefix[x] + (0 if stop_flags[x - 1] else 1)

    # collect + validate every wait on the counter sem
    waiters = []
    for f in nc.m.functions:
        for b in f.blocks:
            for i in b.instructions:
                si = i.sync_info
                if si is None:
                    continue
                for w in si.on_wait:
                    if (w.sync_type, w.id) == skey:
                        if w.wait_mode != "sem-ge-imm" or w.wait_value is None:
                            return 0
                        waiters.append(i)
                        break

    removed = 0
    for i, fl in zip(mms, stop_flags):
        if fl:
            continue
        si = i.sync_info
        new_upd = [u for u in si.on_update if (u.sync_type, u.id) != skey]
        i.sync_info = mybir.SyncInfo(on_wait=list(si.on_wait), on_update=new_upd)
        removed += 1
    for i in waiters:
        si = i.sync_info
        nw = [
            mybir.SyncWait(
                sync_type=w.sync_type,
                id=w.id,
                wait_mode=w.wait_mode,
                ant_name=w.ant_name,
                wait_value=remap(w.wait_value),
                wait_reg=w.wait_reg,
            )
            if (w.sync_type, w.id) == skey
            else w
            for w in si.on_wait
        ]
        i.sync_info = mybir.SyncInfo(on_wait=nw, on_update=list(si.on_update))
    return removed


def _build_program(MT, caps, CIN):
    """Uniform per-core program: MT slots, slot m uses W segment seg(m).

    y[m] = (x[m] @ Wseg) * sc[:, m] for its 2048 columns, fp16 I/O."""
    import concourse.mybir as mybir
    import concourse.tile as tile
    from concourse import bacc

    f32 = mybir.dt.float32
    f16 = mybir.dt.float16

    KT = CIN // P  # 8
    NT = SEG_COLS // N_TILE  # 4
    nseg = sum(1 for q in caps if q > 0)
    seg_of_slot = []
    for s, q in enumerate(caps):
        seg_of_slot += [s] * q

    nc = bacc.Bacc()
    # xt[m, p, k*128+j] = x[token (m,j), cin (k*128+p)] * gate_w(token)
    # (lhsT pretiled, gate weight folded into x on host)
    xt = nc.declare_dram_parameter("xt", [MT, P, CIN], f16, isOutput=False)
    # wt[s, n, p, k, c] = W_seg_s.T[k*128+p, n*512+c]
    wt = nc.declare_dram_parameter(
        "wt", [nseg, NT, P, KT, N_TILE], f16, isOutput=False
    )
    y = nc.declare_dram_parameter("y", [MT, P, SEG_COLS], f16, isOutput=True)

    PF = 6  # x-tile prefetch depth

    with tile.TileContext(nc) as tc:
        with (
            tc.tile_pool(name="wpool", bufs=1) as wpool,
            tc.tile_pool(name="xpool", bufs=min(PF + 2, MT)) as xpool,
            tc.tile_pool(name="opool", bufs=16) as opool,
            tc.tile_pool(name="pspool", bufs=8, space="PSUM") as pspool,
        ):

            # W segments as 3D tiles [p, k, 2048]; one 1MB descriptor per
            # (seg, n-quarter) keeps trigger count low (the trigger itself
            # costs ~600ns of engine time), except the very first n-group
            # which is split per-k so the first psum group can start after
            # ~128KB instead of 1MB.
            wtiles = [
                wpool.tile([P, KT, SEG_COLS], f16, tag=f"w{s}", name=f"w{s}")
                for s in range(nseg)
            ]

            def load_w(s, n, split=None, eng=None):
                if split:
                    # alternate trigger engines: the ~600ns/descriptor issue
                    # cost would otherwise serialize the early W delivery
                    for k in range(KT):
                        e = split[k % 2]
                        e.dma_start(
                            out=wtiles[s][:, k, n * N_TILE : (n + 1) * N_TILE],
                            in_=wt[s, n, :, k],
                        )
                else:
                    (eng or nc.sync).dma_start(
                        out=wtiles[s][:, :, n * N_TILE : (n + 1) * N_TILE],
                        in_=wt[s, n],
                    )

            # x rides gpsimd (idle at start, later interleaves with the
            # y-store triggers), except x0/x1 which gate the first groups
            # and jump the shorter hardware queues; sync carries W so the
            # streams don't queue behind each other.
            def load_xm(m, eng=None):
                xtile = xpool.tile([P, CIN], f16, name="xtile", tag="xtile")
                (eng or nc.gpsimd).dma_start(out=xtile[:], in_=xt[m])
                return xtile

            pending = [load_xm(0, nc.sync)] if MT > 0 else []
            load_w(0, 0, split=(nc.sync, nc.scalar))
            if MT > 1:
                pending.append(load_xm(1, nc.scalar))
            pending += [load_xm(m) for m in range(2, min(PF, MT))]
            load_w(0, 1, split=(nc.sync, nc.scalar))
            load_w(0, 2)
            load_w(0, 3)
            for s in range(1, nseg):
                for n in range(NT):
                    load_w(s, n)

            ydma = nc.scalar

            def emit_group(m, n, xtile):
                s = seg_of_slot[m]
                psum = pspool.tile([P, N_TILE], f32, name="psum", tag="psum")
                for k in range(KT):
                    nc.tensor.matmul(
                        psum[:],
                        lhsT=xtile[:, k * P : (k + 1) * P],
                        rhs=wtiles[s][:, k, n * N_TILE : (n + 1) * N_TILE],
                        start=(k == 0),
                        stop=(k == KT - 1),
                    )
                otile = opool.tile([P, N_TILE], f16)
                nc.vector.tensor_copy(otile[:], psum[:])
                ydma.dma_start(
                    out=y[m, :, n * N_TILE : (n + 1) * N_TILE],
                    in_=otile[:],
                )

            # Ramp: the first R slots all read seg0, so interleave their
            # psum groups n-major -- each W n-chunk is then needed R slots
            # later than with slot-major order, hiding the seg0 delivery
            # ramp behind compute.
            R = max(1, min(6, PF, caps[0], MT - 1))
            xtiles = {m: t for m, t in enumerate(pending)}
            issued = len(xtiles)
            for n in range(NT):
                for m in range(R):
                    emit_group(m, n, xtiles[m])

            for m in range(R, MT):
                s = seg_of_slot[m]
                cnt = 0
                while issued < MT and issued <= m + PF and cnt < 2:
                    xtiles[issued] = load_xm(issued)
                    issued += 1
                    cnt += 1
                xtile = xtiles[m]
                if not LDW_ELIDE or m == MT - 1:
                    # k-inner: the last slot's evictions spread out instead
                    # of bunching into the kernel tail
                    for n in range(NT):
                        emit_group(m, n, xtile)
                else:
                    # n-inner: 4 consecutive matmuls share the stationary
                    # lhsT, making 3/4 of the LDWEIGHTS redundant (removed
                    # by _elide_redundant_ldweights after finalize)
                    psums = [
                        pspool.tile([P, N_TILE], f32, name="psum", tag="psum")
                        for _ in range(NT)
                    ]
                    for k in range(KT):
                        for n in range(NT):
                            nc.tensor.matmul(
                                psums[n][:],
                                lhsT=xtile[:, k * P : (k + 1) * P],
                                rhs=wtiles[s][:, k, n * N_TILE : (n + 1) * N_TILE],
                                start=(k == 0),
                                stop=(k == KT - 1),
                            )
                    for n in range(NT):
                        otile = opool.tile([P, N_TILE], f16)
                        nc.vector.tensor_copy(otile[:], psums[n][:])
                        ydma.dma_start(
                            out=y[m, :, n * N_TILE : (n + 1) * N_TILE],
                            in_=otile[:],
                        )
    nc.finalize()
    if LDW_ELIDE:
        _elide_redundant_ldweights(nc)
    if SEM_ELIDE:
        _elide_matmul_sem_posts(nc)
    return nc


def kernel(x, We, Wg, bg):
    from concourse.bass_utils import run_bass_kernel_spmd

    B, T, CIN = x.shape
    E, DOUT, _ = We.shape
    N = B * T
    x_flat = np.ascontiguousarray(x.reshape(N, CIN), dtype=np.float32)

    top_idx, top_w = _routing(x_flat, Wg, bg)

    # dispatch: token lists per expert
    idx_e, w_e, M = [], [], []
    for e in range(E):
        sel0 = top_idx[:, 0] == e
        sel1 = top_idx[:, 1] == e
        rows = np.nonzero(sel0 | sel1)[0]
        w = np.where(sel0[rows], top_w[rows, 0], top_w[rows, 1]).astype(np.float32)
        idx_e.append(rows)
        w_e.append(w)
        M.append(-(-len(rows) // P))

    MT, caps, fills = _plan(M)
    nseg = sum(1 for q in caps if q > 0)
    seg_base = np.cumsum([0] + list(caps)).tolist()

    WeT16 = np.ascontiguousarray(We.transpose(0, 2, 1)).astype(np.float16)

    KT = CIN // P
    NT = SEG_COLS // N_TILE
    tok = np.zeros((NUM_CORES, MT, P), np.int64)  # token index per slot row
    scf = np.zeros((NUM_CORES, MT, P), np.float32)
    wts = np.zeros((NUM_CORES, nseg, NT, P, KT, N_TILE), np.float16)
    scatter = []  # (core, slot, e, h, count)
    for e, h, lo, hi, core, s in fills:
        rows = idx_e[e]
        ws = w_e[e]
        wts[core, s] = (
            WeT16[e][:, h * SEG_COLS : (h + 1) * SEG_COLS]
            .reshape(KT, P, NT, N_TILE)
            .transpose(2, 1, 0, 3)
        )
        for i, t in enumerate(range(lo, hi)):
            mslot = seg_base[s] + i
            sel = rows[t * P : (t + 1) * P]
            cnt = len(sel)
            if cnt == 0:
                continue
            tok[core, mslot, :cnt] = sel
            scf[core, mslot, :cnt] = ws[t * P : (t + 1) * P]
            scatter.append((core, mslot, e, h, cnt))

    in_maps = []
    for core in range(NUM_CORES):
        # gate weight folded into x (fp16 is a relative format, so the
        # scaling costs no precision; padding rows have weight 0)
        xg = (
            x_flat[tok[core].reshape(-1)] * scf[core].reshape(-1)[:, None]
        ).astype(np.float16)
        xt = np.ascontiguousarray(
            xg.reshape(MT, P, KT, P).transpose(0, 3, 2, 1)
        ).reshape(MT, P, CIN)
        in_maps.append({"xt": xt, "wt": wts[core]})

    key = (MT, caps, CIN, LDW_ELIDE, SEM_ELIDE)
    if key not in _NC_CACHE:
        _NC_CACHE[key] = _build_program(MT, caps, CIN)
    nc = _NC_CACHE[key]
    res = run_bass_kernel_spmd(nc, in_maps, list(range(NUM_CORES)), trace=TRACE)

    LAST_RUN_INFO.clear()
    LAST_RUN_INFO.update(
        exec_time_ns=res.exec_time_ns,
        mean_exec_time_ns=res.mean_exec_time_ns,
        max_exec_time_core_id=res.max_exec_time_core_id,
        profile_json=res.profile_json,
    )

    out = np.zeros((N, DOUT), np.float32)
    for core, mslot, e, h, cnt in scatter:
        ye = res.results[core]["y"][mslot, :cnt].astype(np.float32)
        rows = tok[core, mslot, :cnt]
        out[rows, h * SEG_COLS : (h + 1) * SEG_COLS] += ye
    return out.reshape(B, T, DOUT)


# revision 51
# speedup vs baseline: 1.0042x; 1.0042x over previous
"""MoE top-2 routed linear (nn_MoELinear) on 8 Trainium2 NeuronCores.

Strategy (load-balanced expert parallelism, fp16):
  - Gating (tiny: [N,1024]x[1024,8] matmul + top-2 + softmax) runs on host
    with jax-CPU, replicating the reference op-for-op so the top-2 decisions
    match the reference bitwise.  Gate weights are folded into x on host
    (x is fp16, a relative format, so the scaling costs no precision).
  - The per-expert work is split into "jobs" = (expert, 2048-col output
    half).  Each core holds up to 4 resident W segments of [1024, 2048]
    (fp16) in SBUF; a job is assigned to one or more (core, segment) cells,
    and its token tiles (128 tokens each) fill that cell's slot range.  A
    small search over segment capacities packs all jobs into the minimum
    uniform slot count MT per core (= ceil(2*sum(M_e)/8) = 33 for balanced
    routing), eliminating the load imbalance of one-expert-per-core
    (MT-equivalent 36) -- the PE stream is the bottleneck, so slots are
    roughly proportional to exec time.
  - Per slot the device runs 4 psum groups x 8 fp16 matmuls (128x128x512,
    fp32 PSUM accumulation), evicts through the vector engine as a plain
    cast to fp16, and stores y via scalar-triggered DMA.  Steady-state
    slots use n-inner matmul order so 3/4 of the LDWEIGHTS reload an
    identical stationary operand; post-finalize BIR passes delete those
    (1056 -> ~320 LDWEIGHTS on HW) and strip the per-matmul completion sem
    posts down to the stop-matmuls (remapping wait thresholds).  Both are
    verified correct; pair time stays ~222ns = 512 stream cycles + ~21
    cycles of PE issue cost, so they only trim queue/sem pressure.  W rides the sync queue as big
    multi-dim descriptors (the ~0.6us/descriptor trigger cost and
    ring-depth throttling otherwise stretch W delivery); the first two
    n-groups are split per-k across sync+scalar so the first slot can start
    after ~128KB, and the first R=6 slots' psum groups interleave n-major
    (they share seg0's chunks, so each chunk is needed R slots later --
    this hid the seg0 delivery ramp entirely: mm-stream gaps 6.8us ->
    1.8us, 259us -> 252.7us).  x tiles prefetch 6 deep on gpsimd.  Engine separation
    matters: any long-waiting descriptor queued on an engine head-of-line
    blocks everything behind it (measured as psum-eviction convoys).
  - Host scatters/accumulates the per-slot fp16 outputs into the final
    fp32 result.  Measured max rel err ~4.4e-4 vs the 2e-2 gate.
"""

import os

import numpy as np

NUM_CORES = 8
NUM_EXPERTS = 8
TOP_K = 2
P = 128  # partitions
N_TILE = 512  # psum free-dim tile (one bank of fp32)
SEG_COLS = 2048  # output columns per W segment (half of d_out)
MAX_SEGS = 4

# enable NTFF tracing (sets LAST_RUN_INFO["exec_time_ns"])
TRACE = os.environ.get("MOE_TRACE", "0") == "1"
# n-inner matmul ordering + redundant-LDWEIGHTS removal
LDW_ELIDE = os.environ.get("MOE_LDW_ELIDE", "1") == "1"
# strip per-matmul completion sem posts (keep only stop-matmuls')
SEM_ELIDE = os.environ.get("MOE_SEM_ELIDE", "1") == "1"
MM_DTYPE = "float16"

LAST_RUN_INFO = {}
_NC_CACHE = {}


def _routing(x_flat, Wg, bg):
    """Replicate the reference gating bitwise on jax-CPU; numpy fallback."""
    try:
        import jax
        import jax.numpy as jnp

        with jax.default_device(jax.devices("cpu")[0]):
            xf = jnp.asarray(x_flat)
            gate_logits = xf @ jnp.asarray(Wg).T + jnp.asarray(bg)
            top_w, top_idx = jax.lax.top_k(gate_logits, TOP_K)
            top_w = jax.nn.softmax(top_w, axis=-1)
            return np.asarray(top_idx), np.asarray(top_w)
    except Exception:
        logits = x_flat @ Wg.T + bg
        top_idx = np.argsort(-logits, axis=1, kind="stable")[:, :TOP_K]
        top_v = np.take_along_axis(logits, top_idx, axis=1)
        e = np.exp(top_v - top_v.max(axis=1, keepdims=True))
        top_w = e / e.sum(axis=1, keepdims=True)
        return top_idx, top_w.astype(np.float32)


def _try_assign(jobs, caps):
    """Greedy pack of jobs (size, e, h) into cells of capacities caps x 8.

    Returns list of (e, h, tile_lo, tile_hi, core, seg) cell fills, or None.
    A job may span several cells (its tiles are split across them)."""
    cells = []  # [remaining?, core, seg, cap]
    for s, q in enumerate(caps):
        if q <= 0:
            continue
        for core in range(NUM_CORES):
            cells.append([q, core, s, False])  # cap, core, seg, used
    fills = []
    for size, e, h in jobs:
        free = [c for c in cells if not c[3]]
        # smallest single cell that fits
        fit = None
        for c in sorted(free, key=lambda c: c[0]):
            if c[0] >= size:
                fit = c
                break
        take = []
        if fit is not None:
            take = [fit]
        else:
            got = 0
            for c in sorted(free, key=lambda c: -c[0]):
                if c in take:
                    continue
                # if a single remaining cell can finish the cover, use the
                # smallest such cell instead of the next-largest
                fin = None
                for c2 in sorted(free, key=lambda c: c[0]):
                    if c2 not in take and c2[0] >= size - got:
                        fin = c2
                        break
                if fin is not None:
                    take.append(fin)
                    got += fin[0]
                    break
                take.append(c)
                got += c[0]
                if got >= size:
                    break
            if got < size:
                return None
        lo = 0
        for c in take:
            c[3] = True
            n = min(c[0], size - lo)
            fills.append((e, h, lo, lo + n, c[1], c[2]))
            lo += n
    return fills


def _plan(M):
    """Choose slot count MT, segment capacities, and the job->cell packing."""
    jobs = []
    for e in range(NUM_EXPERTS):
        if M[e] > 0:
            jobs.append((M[e], e, 0))
            jobs.append((M[e], e, 1))
    jobs.sort(reverse=True)
    total = sum(s for s, _, _ in jobs)
    lb = max(1, -(-total // NUM_CORES))
    for mt in range(lb, 4 * 64 + 1):
        for a in range(-(-mt // MAX_SEGS), mt + 1):
            for b in range(0, min(a, mt - a) + 1):
                rem = mt - a - b
                for c in range(max(0, rem - b), min(b, rem) + 1):
                    dd = rem - c
                    if dd > c:
                        continue
                    caps = (a, b, c, dd)
                    fills = _try_assign(jobs, caps)
                    if fills is not None:
                        return mt, caps, fills
    raise RuntimeError("no feasible packing")


def _elide_redundant_ldweights(nc):
    """Post-finalize BIR pass: drop InstLdweights that reload the identical
    weights AP as the previous one (n-inner matmul runs) and carry no
    waits/updates; mark all matmuls non-self-loading so walrus pairs them
    with the surviving loads."""
    import concourse.mybir as mybir

    removed = 0
    for f in nc.m.functions:
        for b in f.blocks:
            new = []
            loaded = None
            for i in b.instructions:
                tn = type(i).__name__
                if tn == "InstLdweights":
                    key = repr(i.ins[0])
                    if loaded == key and not i.has_wait() and not i.has_update():
                        removed += 1
                        continue
                    loaded = key
                elif tn == "InstMatmult":
                    i.ldweights = False
                new.append(i)
            b.instructions = new
    return removed


def _elide_matmul_sem_posts(nc):
    """Post-finalize BIR pass: matmuls post a completion increment on a
    shared counter sem, but every wait threshold on it lands on an
    accumulation-group stop boundary (PE is in-order).  Keep the post only
    on stop matmuls and remap wait thresholds from mm-counts to
    stop-counts.  Bails out (returns 0) unless the sync graph matches the
    expected shape exactly."""
    import concourse.mybir as mybir

    mms = []
    for f in nc.m.functions:
        for b in f.blocks:
            for i in b.instructions:
                if type(i).__name__ == "InstMatmult":
                    mms.append(i)
    if not mms:
        return 0
    # the counter sem = the one updated by every matmul with sem-inc
    from collections import Counter

    upd = Counter()
    for i in mms:
        for u in i.sync_info.on_update:
            if u.update_mode == "sem-inc" and u.update_value == 1:
                upd[(u.sync_type, u.id)] += 1
    keys = [k for k, c in upd.items() if c == len(mms)]
    if len(keys) != 1:
        return 0
    skey = keys[0]

    stop_flags = [bool(i.stop_tensor_calc) for i in mms]
    prefix = [0]
    for fl in stop_flags:
        prefix.append(prefix[-1] + (1 if fl else 0))

    def remap(x):
        x = min(x, len(mms))
        if x <= 0:
            return 0
        return prefix[x] + (0 if stop_flags[x - 1] else 1)

    # collect + validate every wait on the counter sem
    waiters = []
    for f in nc.m.functions:
        for b in f.blocks:
            for i in b.instructions:
                si = i.sync_info
                if si is None:
                    continue
                for w in si.on_wait:
                    if (w.sync_type, w.id) == skey:
                        if w.wait_mode != "sem-ge-imm" or w.wait_value is None:
                            return 0
                        waiters.append(i)
                        break

    removed = 0
    for i, fl in zip(mms, stop_flags):
        if fl:
            continue
        si = i.sync_info
        new_upd = [u for u in si.on_update if (u.sync_type, u.id) != skey]
        i.sync_info = mybir.SyncInfo(on_wait=list(si.on_wait), on_update=new_upd)
        removed += 1
    for i in waiters:
        si = i.sync_info
        nw = [
            mybir.SyncWait(
                sync_type=w.sync_type,
                id=w.id,
                wait_mode=w.wait_mode,
                ant_name=w.ant_name,
                wait_value=remap(w.wait_value),
                wait_reg=w.wait_reg,
            )
            if (w.sync_type, w.id) == skey
            else w
            for w in si.on_wait
        ]
        i.sync_info = mybir.SyncInfo(on_wait=nw, on_update=list(si.on_update))
    return removed


def _build_program(MT, caps, CIN):
    """Uniform per-core program: MT slots, slot m uses W segment seg(m).

    y[m] = (x[m] @ Wseg) * sc[:, m] for its 2048 columns, fp16 I/O."""
    import concourse.mybir as mybir
    import concourse.tile as tile
    from concourse import bacc

    f32 = mybir.dt.float32
    f16 = mybir.dt.float16

    KT = CIN // P  # 8
    NT = SEG_COLS // N_TILE  # 4
    nseg = sum(1 for q in caps if q > 0)
    seg_of_slot = []
    for s, q in enumerate(caps):
        seg_of_slot += [s] * q

    nc = bacc.Bacc()
    # xt[m, p, k*128+j] = x[token (m,j), cin (k*128+p)] * gate_w(token)
    # (lhsT pretiled, gate weight folded into x on host)
    xt = nc.declare_dram_parameter("xt", [MT, P, CIN], f16, isOutput=False)
    # wt[s, n, p, k, c] = W_seg_s.T[k*128+p, n*512+c]
    wt = nc.declare_dram_parameter(
        "wt", [nseg, NT, P, KT, N_TILE], f16, isOutput=False
    )
    y = nc.declare_dram_parameter("y", [MT, P, SEG_COLS], f16, isOutput=True)

    PF = 6  # x-tile prefetch depth

    with tile.TileContext(nc) as tc:
        with (
            tc.tile_pool(name="wpool", bufs=1) as wpool,
            tc.tile_pool(name="xpool", bufs=min(PF + 2, MT)) as xpool,
            tc.tile_pool(name="opool", bufs=16) as opool,
            tc.tile_pool(name="pspool", bufs=8, space="PSUM") as pspool,
        ):

            # W segments as 3D tiles [p, k, 2048]; one 1MB descriptor per
            # (seg, n-quarter) keeps trigger count low (the trigger itself
            # costs ~600ns of engine time), except the very first n-group
            # which is split per-k so the first psum group can start after
            # ~128KB instead of 1MB.
            wtiles = [
                wpool.tile([P, KT, SEG_COLS], f16, tag=f"w{s}", name=f"w{s}")
                for s in range(nseg)
            ]

            def load_w(s, n, split=None, eng=None):
                if split:
                    # alternate trigger engines: the ~600ns/descriptor issue
                    # cost would otherwise serialize the early W delivery
                    for k in range(KT):
                        e = split[k % 2]
                        e.dma_start(
                            out=wtiles[s][:, k, n * N_TILE : (n + 1) * N_TILE],
                            in_=wt[s, n, :, k],
                        )
                else:
                    (eng or nc.sync).dma_start(
                        out=wtiles[s][:, :, n * N_TILE : (n + 1) * N_TILE],
                        in_=wt[s, n],
                    )

            # x rides gpsimd (idle at start, later interleaves with the
            # y-store triggers); sync carries only W + scales so neither
            # stream queues behind the other.
            def load_xm(m):
                xtile = xpool.tile([P, CIN], f16, name="xtile", tag="xtile")
                nc.gpsimd.dma_start(out=xtile[:], in_=xt[m])
                return xtile

            load_w(0, 0, split=(nc.sync, nc.scalar))
            pending = [load_xm(m) for m in range(min(PF, MT))]
            load_w(0, 1, split=(nc.sync, nc.scalar))
            load_w(0, 2)
            load_w(0, 3)
            for s in range(1, nseg):
                for n in range(NT):
                    load_w(s, n)

            ydma = nc.scalar

            def emit_group(m, n, xtile):
                s = seg_of_slot[m]
                psum = pspool.tile([P, N_TILE], f32, name="psum", tag="psum")
                for k in range(KT):
                    nc.tensor.matmul(
                        psum[:],
                        lhsT=xtile[:, k * P : (k + 1) * P],
                        rhs=wtiles[s][:, k, n * N_TILE : (n + 1) * N_TILE],
                        start=(k == 0),
                        stop=(k == KT - 1),
                    )
                otile = opool.tile([P, N_TILE], f16)
                nc.vector.tensor_copy(otile[:], psum[:])
                ydma.dma_start(
                    out=y[m, :, n * N_TILE : (n + 1) * N_TILE],
                    in_=otile[:],
                )

            # Ramp: the first R slots all read seg0, so interleave their
            # psum groups n-major -- each W n-chunk is then needed R slots
            # later than with slot-major order, hiding the seg0 delivery
            # ramp behind compute.
            R = max(1, min(6, PF, caps[0], MT - 1))
            xtiles = {m: t for m, t in enumerate(pending)}
            issued = len(xtiles)
            for n in range(NT):
                for m in range(R):
                    emit_group(m, n, xtiles[m])

            for m in range(R, MT):
                s = seg_of_slot[m]
                cnt = 0
                while issued < MT and issued <= m + PF and cnt < 2:
                    xtiles[issued] = load_xm(issued)
                    issued += 1
                    cnt += 1
                xtile = xtiles[m]
                if not LDW_ELIDE or m == MT - 1:
                    # k-inner: the last slot's evictions spread out instead
                    # of bunching into the kernel tail
                    for n in range(NT):
                        emit_group(m, n, xtile)
                else:
                    # n-inner: 4 consecutive matmuls share the stationary
                    # lhsT, making 3/4 of the LDWEIGHTS redundant (removed
                    # by _elide_redundant_ldweights after finalize)
                    psums = [
                        pspool.tile([P, N_TILE], f32, name="psum", tag="psum")
                        for _ in range(NT)
                    ]
                    for k in range(KT):
                        for n in range(NT):
                            nc.tensor.matmul(
                                psums[n][:],
                                lhsT=xtile[:, k * P : (k + 1) * P],
                                rhs=wtiles[s][:, k, n * N_TILE : (n + 1) * N_TILE],
                                start=(k == 0),
                                stop=(k == KT - 1),
                            )
                    for n in range(NT):
                        otile = opool.tile([P, N_TILE], f16)
                        nc.vector.tensor_copy(otile[:], psums[n][:])
                        ydma.dma_start(
                            out=y[m, :, n * N_TILE : (n + 1) * N_TILE],
                            in_=otile[:],
                        )
    nc.finalize()
    if LDW_ELIDE:
        _elide_redundant_ldweights(nc)
    if SEM_ELIDE:
        _elide_matmul_sem_posts(nc)
    return nc


def kernel(x, We, Wg, bg):
    from concourse.bass_utils import run_bass_kernel_spmd

    B, T, CIN = x.shape
    E, DOUT, _ = We.shape
    N = B * T
    x_flat = np.ascontiguousarray(x.reshape(N, CIN), dtype=np.float32)

    top_idx, top_w = _routing(x_flat, Wg, bg)

    # dispatch: token lists per expert
    idx_e, w_e, M = [], [], []
    for e in range(E):
        sel0 = top_idx[:, 0] == e
        sel1 = top_idx[:, 1] == e
        rows = np.nonzero(sel0 | sel1)[0]
        w = np.where(sel0[rows], top_w[rows, 0], top_w[rows, 1]).astype(np.float32)
        idx_e.append(rows)
        w_e.append(w)
        M.append(-(-len(rows) // P))

    MT, caps, fills = _plan(M)
    nseg = sum(1 for q in caps if q > 0)
    seg_base = np.cumsum([0] + list(caps)).tolist()

    WeT16 = np.ascontiguousarray(We.transpose(0, 2, 1)).astype(np.float16)

    KT = CIN // P
    NT = SEG_COLS // N_TILE
    tok = np.zeros((NUM_CORES, MT, P), np.int64)  # token index per slot row
    scf = np.zeros((NUM_CORES, MT, P), np.float32)
    wts = np.zeros((NUM_CORES, nseg, NT, P, KT, N_TILE), np.float16)
    scatter = []  # (core, slot, e, h, count)
    for e, h, lo, hi, core, s in fills:
        rows = idx_e[e]
        ws = w_e[e]
        wts[core, s] = (
            WeT16[e][:, h * SEG_COLS : (h + 1) * SEG_COLS]
            .reshape(KT, P, NT, N_TILE)
            .transpose(2, 1, 0, 3)
        )
        for i, t in enumerate(range(lo, hi)):
            mslot = seg_base[s] + i
            sel = rows[t * P : (t + 1) * P]
            cnt = len(sel)
            if cnt == 0:
                continue
            tok[core, mslot, :cnt] = sel
            scf[core, mslot, :cnt] = ws[t * P : (t + 1) * P]
            scatter.append((core, mslot, e, h, cnt))

    in_maps = []
    for core in range(NUM_CORES):
        # gate weight folded into x (fp16 is a relative format, so the
        # scaling costs no precision; padding rows have weight 0)
        xg = (
            x_flat[tok[core].reshape(-1)] * scf[core].reshape(-1)[:, None]
        ).astype(np.float16)
        xt = np.ascontiguousarray(
            xg.reshape(MT, P, KT, P).transpose(0, 3, 2, 1)
        ).reshape(MT, P, CIN)
        in_maps.append({"xt": xt, "wt": wts[core]})

    key = (MT, caps, CIN, LDW_ELIDE, SEM_ELIDE)
    if key not in _NC_CACHE:
        _NC_CACHE[key] = _build_program(MT, caps, CIN)
    nc = _NC_CACHE[key]
    res = run_bass_kernel_spmd(nc, in_maps, list(range(NUM_CORES)), trace=TRACE)

    LAST_RUN_INFO.clear()
    LAST_RUN_INFO.update(
        exec_time_ns=res.exec_time_ns,
        mean_exec_time_ns=res.mean_exec_time_ns,
        max_exec_time_core_id=res.max_exec_time_core_id,
        profile_json=res.profile_json,
    )

    out = np.zeros((N, DOUT), np.float32)
    for core, mslot, e, h, cnt in scatter:
        ye = res.results[core]["y"][mslot, :cnt].astype(np.float32)
        rows = tok[core, mslot, :cnt]
        out[rows, h * SEG_COLS : (h + 1) * SEG_COLS] += ye
    return out.reshape(B, T, DOUT)
